# revision 78
# baseline (speedup 1.0000x reference)
"""MultiHeadSelfAttention Trainium2 Bass kernel (v3).

Shapes (hardcoded): B=8, N=2048, E=512, H=8 heads, D=64 head dim.
Sharding: data-parallel over batch -> one batch item per NeuronCore (8 cores),
no collectives needed.

Per-core pipeline (bf16 compute, fp32 accumulate):
  prologue (DMA-chasing order Wk,K,Wq,Q0,Wv,V): load f32, cast bf16
        (DVE/ACT), PE-transpose into ^T layouts. k-proj emits kTT[h] =
        [128, N/2]: partitions 0-63 hold head h's d-dims for EVEN key
        tiles, partitions 64-127 for ODD key tiles (col-tiled matmul
        pair with interleaved rhs). q-proj emits qTT[h] with q^T
        duplicated in both partition halves (col-tiled pair) so the
        scores matmuls are ROW-TILED: the two 64-row score matmuls of an
        adjacent key-tile pair run concurrently in the upper/lower
        halves of the PE array. v_aug = V @ Wv^T + per-head ones column
        (softmax denominators ride the PV matmul). Only K/Q-chunk-0/
        V-groups-0,1 gate attention start; the rest streams in as filler.
  attention (qc outer, head inner): per key-tile pair, s2 [128, 2x512] =
        row-tiled score pair; exp on ScalarE for 7/8 pairs, on VectorE
        via a Schraudolph exponent trick (affine -> int16 -> bits
        reinterpreted as bf16) for 1/8. PV is emitted with lag 2 so the
        exp latency is off the critical path: O^T [65, 512] += v_aug^T .
        P^T over 16 key tiles; row 64 = softmax denominators. Remaining
        projections / transposes / previous-chunk output projection are
        interleaved as PE filler (one unit per key-tile pair) to keep
        the HAM clock gate at full rate.
  per-head-pair: denominator row -> DRAM -> broadcast -> fast reciprocal
        (DVE), normalize O^T on gpsimd. After each chunk's last head the
        output projection Y = oTn^T . Wo^T is queued (computed directly
        in [n, e] orientation -- no output transposes), bias-add fused
        into the PSUM->SBUF copy, DMA out; all overlapped with the next
        chunk's attention.

The attention mask is all ones per the problem spec; validated host-side.
"""

import sys

for _p in ("/opt/trn_rl_repo",):
    if _p not in sys.path:
        sys.path.insert(0, _p)

import numpy as np
from collections import deque
from contextlib import ExitStack

import concourse.bass as bass
import concourse.bacc as bacc
import concourse.mybir as mybir
import concourse.tile as tile
from concourse.masks import make_identity

B, N, E = 8, 2048, 512
H, D = 8, 64
P = 128          # partitions
ET = E // P      # 4 e-tiles
NT = N // P      # 16 n-tiles
QC = 512         # q chunk in attention
NQC = N // QC    # 4
KTP = 8          # key-tile pairs; pair k covers key tiles 2k and 2k+1
HV = 65          # head dim + ones column
PVLAG = 3        # PV trails scores by this many key-tile pairs
FP32 = mybir.dt.float32
BF16 = mybir.dt.bfloat16
I16 = mybir.dt.int16
NCORES = 8

# which key-tile pairs' exp goes to the DVE Schraudolph path (rest on ACT)
SCH_KTPS = (2, 6)
# Schraudolph: exp(x) ~ bf16(bits = int16(x*log2(e)*128 + 127*128 + SIGMA))
SCH_SIGMA = -5.0

AF = mybir.ActivationFunctionType
ALU = mybir.AluOpType


def _build(inv_tau: float) -> bass.Bass:
    nc = bacc.Bacc(trn_type="TRN2")

    dQ = nc.dram_tensor("Q", [N, E], FP32, kind="ExternalInput")
    dK = nc.dram_tensor("K", [N, E], FP32, kind="ExternalInput")
    dV = nc.dram_tensor("V", [N, E], FP32, kind="ExternalInput")
    dWq = nc.dram_tensor("Wq", [E, E], FP32, kind="ExternalInput")
    dWk = nc.dram_tensor("Wk", [E, E], FP32, kind="ExternalInput")
    dWv = nc.dram_tensor("Wv", [E, E], FP32, kind="ExternalInput")
    dWo = nc.dram_tensor("Wo", [E, E], FP32, kind="ExternalInput")
    dbo = nc.dram_tensor("bo", [E], FP32, kind="ExternalInput")
    dout = nc.dram_tensor("out", [N, E], FP32, kind="ExternalOutput")
    drs = nc.dram_tensor("r_scratch", [NQC * H * QC], FP32)

    with tile.TileContext(nc) as tc, ExitStack() as ctx:
        _body(ctx, tc, inv_tau, dQ, dK, dV, dWq, dWk, dWv, dWo, dbo, dout, drs)
    nc.finalize()
    return nc


def _body(ctx, tc, inv_tau, dQ, dK, dV, dWq, dWk, dWv, dWo, dbo, dout, drs):
    nc = tc.nc
    dma = nc.sync.dma_start

    const = ctx.enter_context(tc.tile_pool(name="const", bufs=1))
    big = ctx.enter_context(tc.tile_pool(name="big", bufs=1))
    psum = ctx.enter_context(tc.tile_pool(name="psum", bufs=1, space="PSUM"))
    stage = ctx.enter_context(tc.tile_pool(name="stage", bufs=1))
    p2pool = ctx.enter_context(tc.tile_pool(name="p2pool", bufs=4))

    ident = const.tile([P, P], BF16, name="ident", tag="ident")
    make_identity(nc, ident)

    in_attention = [False]

    # ---------------- helpers: loads, casts, transposes, drains ----------
    cast_rr = [0]

    def load_cast(dX, r, fast=False, q2=False):
        """DMA [128,E] f32 slice r, cast to bf16.

        Prologue (or fast=True): DVE/ACT alternating (fast engines, idle
        then). Attention phase: gpsimd (slow but otherwise idle).
        q2: use the second HWDGE queue (scalar) + gpsimd/DVE casts, so V
        streams in parallel with K/Q instead of behind them."""
        x_f32 = stage.tile([P, E], FP32, name="x_f32", tag="x_f32", bufs=6)
        if q2:
            nc.scalar.dma_start(out=x_f32, in_=dX[r * P:(r + 1) * P, :])
            x_bf = stage.tile([P, E], BF16, name="x_bf", tag="x_bf", bufs=8)
            cast_rr[0] ^= 1
            if cast_rr[0]:
                nc.gpsimd.tensor_copy(x_bf, x_f32)
            else:
                nc.vector.tensor_copy(x_bf, x_f32)
            return x_bf
        dma(out=x_f32, in_=dX[r * P:(r + 1) * P, :])
        x_bf = stage.tile([P, E], BF16, name="x_bf", tag="x_bf", bufs=8)
        if in_attention[0] and not fast:
            nc.gpsimd.tensor_copy(x_bf, x_f32)
        else:
            cast_rr[0] ^= 1
            if cast_rr[0]:
                nc.vector.tensor_copy(x_bf, x_f32)
            else:
                nc.scalar.copy(x_bf, x_f32)
        return x_bf

    tp_rr = [0]

    def tp_tile():
        """PSUM staging tile for transposes.

        Prologue rotates over the (idle) s2/o2 tags; during attention the
        pp tag is used instead (s2/o2 are hot in the score/PV pipeline).
        """
        if in_attention[0]:
            return psum.tile([P, E], BF16, name="tp", tag="pp", bufs=2)
        tp_rr[0] ^= 1
        if tp_rr[0]:
            return psum.tile([P, E], BF16, name="tp", tag="s2", bufs=2)
        return psum.tile([P, E], BF16, name="tp", tag="o2", bufs=2)

    copy_rr = [0]

    def drain_copy(out_ap, in_ap):
        """PSUM->SBUF copy, DVE/ACT alternating (ACT has headroom in the
        filler-heavy chunks where these copies bunch up)."""
        copy_rr[0] ^= 1
        if copy_rr[0]:
            nc.scalar.copy(out_ap, in_ap)
        else:
            nc.vector.tensor_copy(out_ap, in_ap)

    # ---------------- SBUF layout ----------------
    # W^T / X^T live in merged tiles (so one XBAR DMA-transpose can write
    # all four e-blocks of an input tile); consumers use slice views.
    # wt[w][c] = [128 in-dims (block c), 512 out-dims]
    # xT[x][et] = [128 e-dims (block et), N]
    wtbig = {w: const.tile([P, ET * E], BF16, name=f"wT_{w}", tag=f"wT_{w}")
             for w in ("q", "k", "v", "o")}
    wt = {w: [wtbig[w][:, c * E:(c + 1) * E] for c in range(ET)]
          for w in ("q", "k", "v", "o")}
    xTbig = {x: big.tile([P, ET * N], BF16, name=f"xT_{x}", tag=f"xT_{x}")
             for x in ("K", "V", "Q")}
    xT = {x: [xTbig[x][:, et * N:(et + 1) * N] for et in range(ET)]
          for x in ("K", "V", "Q")}

    def dma_transpose_tile(dest_big, r, x_bf):
        """One XBAR transpose: x_bf [128, 512] -> 128-col block r of each
        of dest's 4 e-blocks. Used in the attention phase only (the sync
        DMA queue is quiet there; PE is not)."""
        dv = dest_big.rearrange("p (b n) -> p b n", b=ET)
        nc.sync.dma_start_transpose(out=dv[:, :, r * P:(r + 1) * P],
                                    in_=x_bf)
    # kTT[h]: [128, N/2]; col block k = key-tile pair (2k, 2k+1):
    # partitions 0-63 = head h d-dims of tile 2k, 64-127 = tile 2k+1
    kTT = [big.tile([P, N // 2], BF16, name=f"kTT_{h}", tag=f"kTT_{h}")
           for h in range(H)]
    # qTT[h]: [128, N]; q^T duplicated into both partition halves
    qTT = [big.tile([P, N], BF16, name=f"qTT_{h}", tag=f"qTT_{h}")
           for h in range(H)]
    v_aug = [big.tile([P, H * HV], BF16, name=f"vaug_{nt}",
                      tag=f"vaug_{nt}") for nt in range(NT)]

    def ham_heartbeat():
        """One real (non-transpose) matmul: PE-transposes don't count as
        busy for the HAM clock gate, so transpose-heavy stretches would
        re-throttle the PE to 1.2 GHz without these. Needs to fire at
        least every ~2us (the re-throttle window is ~3.4us and a cold
        transpose half-group takes ~2us)."""
        wh = psum.tile([P, P], FP32, name="warm", tag="pp", bufs=2)
        nc.tensor.matmul(wh, lhsT=ident, rhs=ident, start=True, stop=True,
                         skip_group_check=True)

    def emit_w_block(wname, dW, fast=False):
        """Load + cast + transpose one weight matrix into wt[wname]."""
        w_bfs = [load_cast(dW, r, fast) for r in range(ET)]
        for c in range(ET):
            tp = tp_tile()
            for r in range(ET):
                nc.tensor.transpose(
                    tp[:, r * P:(r + 1) * P], w_bfs[r][:, c * P:(c + 1) * P],
                    ident)
            drain_copy(wt[wname][c], tp)
            if not in_attention[0] and c % 2 == 1:
                ham_heartbeat()

    def emit_x_casts(dX, g, fast=False):
        """DMA + cast one group of 4 n-tiles; returns staged bf16 tiles."""
        return [load_cast(dX, g * ET + i, fast) for i in range(ET)]

    def emit_x_transposes(xname, g, x_bfs):
        for et in range(ET):
            tp = tp_tile()
            for i in range(ET):
                nc.tensor.transpose(
                    tp[:, i * P:(i + 1) * P],
                    x_bfs[i][:, et * P:(et + 1) * P], ident)
            drain_copy(xT[xname][et][:, g * E:(g + 1) * E], tp)
            if not in_attention[0] and et % 2 == 1:
                ham_heartbeat()

    def emit_x_group(xname, dX, g, fast=False):
        emit_x_transposes(xname, g, emit_x_casts(dX, g, fast))

    # ---------------- projections ----------------
    def emit_kproj(h, c):
        """col-tiled pair: even key tiles (upper half) + odd (lower half).

        kTT chunk c covers key-tile pairs 4c..4c+3 = tiles 8c..8c+7."""
        pp = psum.tile([P, QC], FP32, name="pp", tag="pp", bufs=2)
        lhs = [wt["k"][et][:, h * D:(h + 1) * D] for et in range(ET)]
        for half in range(2):
            for et in range(ET):
                xk = xT["K"][et].rearrange("p (g u c2) -> p g u c2",
                                           u=2, c2=P)
                nc.tensor.matmul(
                    pp[half * D:(half + 1) * D, :], lhsT=lhs[et],
                    rhs=xk[:, 4 * c:4 * (c + 1), half, :],
                    start=(et == 0), stop=(et == ET - 1))
        drain_copy(kTT[h][:, c * QC:(c + 1) * QC], pp)

    def emit_qproj(h, c):
        """col-tiled pair: same q^T chunk written to both partition halves."""
        pp = psum.tile([P, QC], FP32, name="pp", tag="pp", bufs=2)
        lhs = [wt["q"][et][:, h * D:(h + 1) * D] for et in range(ET)]
        rhs = [xT["Q"][et][:, c * QC:(c + 1) * QC] for et in range(ET)]
        for half in range(2):
            for et in range(ET):
                nc.tensor.matmul(pp[half * D:(half + 1) * D, :],
                                 lhsT=lhs[et], rhs=rhs[et],
                                 start=(et == 0), stop=(et == ET - 1))
        drain_copy(qTT[h][:, c * QC:(c + 1) * QC], pp)

    def emit_vproj(nt):
        pp = psum.tile([P, QC], FP32, name="pp", tag="pp", bufs=2)
        for et in range(ET):
            nc.tensor.matmul(
                pp,
                lhsT=xT["V"][et][:, nt * P:(nt + 1) * P],
                rhs=wt["v"][et],
                start=(et == 0), stop=(et == ET - 1))
        va = v_aug[nt].rearrange("p (h c) -> p h c", c=HV)
        nc.vector.tensor_copy(va[:, :, 0:D], pp.rearrange("p (h d) -> p h d",
                                                          d=D))
        nc.gpsimd.memset(va[:, :, D:HV], 1.0)

    def emit_v_unit(g, x_bfs):
        emit_x_transposes("V", g, x_bfs)
        for nt in range(g * ET, (g + 1) * ET):
            emit_vproj(nt)

    # ---------------- prologue emission (DMA-chasing order) ----------------
    # HAM warmup: dummy matmuls while the first DMAs land (PE would idle
    # anyway; sustained activity lifts the clock gate to 2.4 GHz before
    # the real transposes start). One shared PSUM tile, back-to-back
    # matmuls -- buffer rotation would add gaps and dilute the busy window.
    warm = psum.tile([P, P], FP32, name="warm", tag="pp", bufs=2)
    for _ in range(48):
        nc.tensor.matmul(warm, lhsT=ident, rhs=ident, start=True, stop=True,
                         skip_group_check=True)
    # K group 0 first in the DMA queue (its transposes only need ident).
    # kproj bursts are interleaved between transpose groups: dense matmuls
    # count as PE-busy for the HAM clock gate (transposes don't), keeping
    # the prologue at 2.4 GHz.
    k0_staged = emit_x_casts(dK, 0)
    # bias broadcast to all partitions (consumed only by oproj; queued
    # after the first K tiles so it never delays them)
    bo_sb = const.tile([P, E], FP32, name="bo_sb", tag="bo_sb")
    dma(out=bo_sb, in_=bass.AP(tensor=dbo, offset=0, ap=[[0, P], [1, E]]))
    emit_w_block("k", dWk)
    emit_x_transposes("K", 0, k0_staged)
    for g in range(1, NT // ET):
        emit_x_group("K", dK, g)
    emit_kproj(0, 0)
    emit_kproj(0, 1)
    emit_w_block("q", dWq)
    emit_x_group("Q", dQ, 0)
    emit_qproj(0, 0)
    emit_w_block("v", dWv)
    for g in range(2):
        emit_v_unit(g, emit_x_casts(dV, g))
    # V groups 2,3: DMA+cast now (fast engines), transposes+proj as filler
    v_staged = {g: emit_x_casts(dV, g, fast=True) for g in (2, 3)}

    # ---------------- attention ----------------
    sch_scale = float(inv_tau * np.log2(np.e) * 128.0)
    sch_bias = float(127 * 128 + SCH_SIGMA)

    filler = deque()

    def pop_filler():
        if filler:
            fn, args = filler.popleft()
            fn(*args)

    # seed: per-head k/q projections must be emitted before that head's
    # scores. qc0 pops 4 units per head (ktp 1,3,5,7); the order below is
    # tuned so every unit lands before its first consumer.
    filler.append((emit_v_unit, (2, v_staged[2])))
    filler.append((emit_kproj, (1, 0)))
    filler.append((emit_qproj, (1, 0)))
    filler.append((emit_v_unit, (3, v_staged[3])))
    filler.append((emit_kproj, (1, 1)))
    for h in range(2, H):
        filler.append((emit_kproj, (h, 0)))
        filler.append((emit_qproj, (h, 0)))
        filler.append((emit_kproj, (h, 1)))
        if h in (3, 4, 5):
            filler.append((emit_x_group, ("Q", dQ, h - 2)))
        if h == 6:
            filler.append((emit_w_block, ("o", dWo)))

    def _drs_off(qc, h):
        return qc * H * QC + (h % 4) * 2 * QC + (h // 4) * QC

    def emit_pv(h, o2, p2, ktp):
        for j in range(2):
            kt = 2 * ktp + j
            nc.tensor.matmul(
                o2[:, :],
                lhsT=v_aug[kt][:, h * HV:(h + 1) * HV],
                rhs=p2[:, j * QC:(j + 1) * QC],
                start=(ktp == 0 and j == 0), stop=(ktp == KTP - 1 and j == 1),
                skip_group_check=True)

    rbr_live = {}
    pend = deque()  # (h, o2, p2, ktp, drain_ctx) PVs awaiting emission

    def emit_drains(h, qc, oT_t, oTn_t, l4_t, o2):
        """After head h's last PV: O^T/denominator drains + normalization.

        Drains go to DVE except for the last chunk's final head pair,
        whose tail chain is latency-critical while ACT is winding down
        (earlier qc3 heads stay on DVE -- ACT is the co-pacer there)."""
        drain = (nc.scalar.copy if qc == NQC - 1 and h >= H - 2
                 else nc.vector.tensor_copy)
        hp, h2 = h // 2, (h % 2) * D
        drain(oT_t[hp][h2:h2 + D, :], o2[0:D, :])
        g = 32 * (h % 4)
        l4_slice = l4_t[g:g + 1, (h // 4) * QC:(h // 4 + 1) * QC]
        drain(l4_slice, o2[D:HV, :])
        dma(out=drs[_drs_off(qc, h):_drs_off(qc, h) + QC], in_=l4_slice)
        # broadcast this head's denominators to its 64 partitions
        if h % 2 == 0:
            rbr_live[hp] = stage.tile([P, QC], FP32, name="rbr", tag="rbr",
                                      bufs=2)
        rbr = rbr_live[hp]
        bsrc = bass.AP(tensor=drs, offset=_drs_off(qc, h),
                       ap=[[0, D], [1, QC]])
        dma(out=rbr[h2:h2 + D, :], in_=bsrc)
        if h % 2 == 1:
            # head pair complete: reciprocal + normalize (DVE, so the
            # gpsimd drain queue never blocks on the reciprocal)
            rb = stage.tile([P, QC], FP32, name="rb", tag="rb", bufs=2)
            nc.vector.reciprocal_approx_fast(rb, rbr)
            nc.vector.tensor_tensor(oTn_t[hp], oT_t[hp], rb, ALU.mult)

    def flush_pv():
        h, o2, p2, ktp, drain_ctx = pend.popleft()
        emit_pv(h, o2, p2, ktp)
        if ktp == KTP - 1:
            emit_drains(h, *drain_ctx, o2)

    def attn_head(h, qc, oT_t, oTn_t, l4_t):
        o2 = psum.tile([HV, QC], FP32, name="o2", tag="o2", bufs=2)
        drain_ctx = (qc, oT_t, oTn_t, l4_t)
        for ktp in range(KTP):
            s2 = psum.tile([P, 2 * QC], FP32, name="s2", tag="s2", bufs=2)
            # row-tiled score pair: upper 64 rows = key tile 2*ktp,
            # lower 64 rows = key tile 2*ktp+1
            nc.tensor.matmul(
                s2[:, 0:QC],
                lhsT=kTT[h][0:D, ktp * P:(ktp + 1) * P],
                rhs=qTT[h][0:D, qc * QC:(qc + 1) * QC],
                start=True, stop=True)
            nc.tensor.matmul(
                s2[:, QC:2 * QC],
                lhsT=kTT[h][D:P, ktp * P:(ktp + 1) * P],
                rhs=qTT[h][D:P, qc * QC:(qc + 1) * QC],
                start=True, stop=True)
            p2 = p2pool.tile([P, 2 * QC], BF16, name="p2", tag="p2")
            if ktp in SCH_KTPS:
                nc.vector.tensor_scalar(
                    p2.bitcast(I16), s2, sch_scale, sch_bias,
                    ALU.mult, ALU.add)
            else:
                nc.scalar.activation(p2, s2, AF.Exp, scale=inv_tau)
            pend.append((h, o2, p2, ktp, drain_ctx))
            if len(pend) > PVLAG:
                flush_pv()
            # qc0 pops every other pair (its filler backlog is the whole
            # projection prologue; pacing avoids a PE bulge), later
            # chunks pop every pair. The last chunk's first pops must wait
            # for the previous chunk's PV pipeline (and with it head 7's
            # normalization) to flush out -- its oproj units lead the queue.
            if qc > 0 and h == 0 and ktp < 4:
                pass  # previous chunk's PV pipeline still flushing; its
                # oproj units lead the queue and need head 7's norm
            elif qc > 0 or ktp % 2 == 1:
                pop_filler()

    def emit_oproj(qc, j, oTn_t, phase=0, pp_store=None):
        """y rows nt=qc*4+j: out [128 n, 512 e] = oTn^T blocks . Wo^T.

        phase 0: whole thing. phase 1: e-blocks 0-2 only (available before
        the last head pair is normalized -- fills the tail's norm-chain
        latency). phase 2: e-block 3 + drain + store."""
        if phase != 2:
            tag = "pp" if phase == 0 or j < 2 else ("o2" if j == 2 else "s2")
            pp = psum.tile([P, QC], FP32, name="pp", tag=tag, bufs=2)
            lim = ET if phase == 0 else ET - 1
            for et in range(lim):
                nc.tensor.matmul(
                    pp,
                    lhsT=oTn_t[et][:, j * P:(j + 1) * P],
                    rhs=wt["o"][et],
                    start=(et == 0), stop=(phase == 0 and et == ET - 1),
                    skip_group_check=True)
            if phase == 1:
                pp_store[j] = pp
                return
        else:
            pp = pp_store[j]
            nc.tensor.matmul(
                pp,
                lhsT=oTn_t[ET - 1][:, j * P:(j + 1) * P],
                rhs=wt["o"][ET - 1],
                start=False, stop=True, skip_group_check=True)
        y_sb = stage.tile([P, E], FP32, name="y_sb", tag="y_sb", bufs=2)
        nc.vector.tensor_tensor(y_sb, pp, bo_sb, ALU.add)
        nt = qc * NQC + j
        dma(out=dout[nt * P:(nt + 1) * P, :], in_=y_sb)

    in_attention[0] = True
    for qc in range(NQC):
        l4_t = stage.tile([P, 2 * QC], FP32, name="l4", tag="l4", bufs=2)
        oT_t = [big.tile([P, QC], BF16, name=f"oT_{hp}", tag=f"oT_{hp}",
                         bufs=2) for hp in range(ET)]
        oTn_t = [big.tile([P, QC], BF16, name=f"oTn_{hp}", tag=f"oTn_{hp}",
                          bufs=2) for hp in range(ET)]
        if qc + 1 < NQC:
            for h in range(H):
                filler.append((emit_qproj, (h, qc + 1)))
        for h in range(H):
            attn_head(h, qc, oT_t, oTn_t, l4_t)
        # The PV pipeline spills across the chunk boundary (flushing here
        # would idle the PE on the last exp). This chunk's oproj units are
        # queued behind the next chunk's qproj units, so head 7's drains
        # (emitted when its last PV pops out early next chunk) land first.
        if qc + 1 < NQC:
            for j in range(NQC):
                filler.append((emit_oproj, (qc, j, oTn_t)))
        else:
            while pend:
                flush_pv()
            while filler:
                pop_filler()
            for j in range(NQC):
                emit_oproj(qc, j, oTn_t)


_CACHE = {}


def _get_nc(inv_tau: float) -> bass.Bass:
    key = round(float(inv_tau), 9)
    if key not in _CACHE:
        _CACHE[key] = _build(float(inv_tau))
    return _CACHE[key]


def _run(inputs: dict, trace: bool = False):
    """Returns (output [B,N,E] fp32, BassKernelResults)."""
    from concourse.bass_utils import run_bass_kernel_spmd

    Q = np.ascontiguousarray(np.asarray(inputs["Q"], dtype=np.float32))
    K = np.ascontiguousarray(np.asarray(inputs["K"], dtype=np.float32))
    V = np.ascontiguousarray(np.asarray(inputs["V"], dtype=np.float32))
    Wq = np.ascontiguousarray(np.asarray(inputs["Wq"], dtype=np.float32))
    Wk = np.ascontiguousarray(np.asarray(inputs["Wk"], dtype=np.float32))
    Wv = np.ascontiguousarray(np.asarray(inputs["Wv"], dtype=np.float32))
    Wo = np.ascontiguousarray(np.asarray(inputs["Wo"], dtype=np.float32))
    bo = np.ascontiguousarray(np.asarray(inputs["bo"], dtype=np.float32))
    tau = float(np.asarray(inputs["tau"]))

    mask = inputs.get("attn_mask")
    if mask is not None and not np.all(np.asarray(mask) != 0):
        # Fallback (never hit for the spec'd all-ones mask): host math.
        return _host_reference(Q, K, V, np.asarray(mask), Wq, Wk, Wv, Wo,
                               bo, tau), None

    nc = _get_nc(1.0 / tau)
    in_maps = []
    for b in range(NCORES):
        in_maps.append({
            "Q": Q[b], "K": K[b], "V": V[b],
            "Wq": Wq, "Wk": Wk, "Wv": Wv, "Wo": Wo, "bo": bo,
        })
    res = run_bass_kernel_spmd(nc, in_maps, list(range(NCORES)), trace=trace)
    out = np.stack([np.asarray(res.results[b]["out"]) for b in range(NCORES)])
    return out.astype(np.float32), res


def _host_reference(Q, K, V, mask, Wq, Wk, Wv, Wo, bo, tau):
    b, n, _ = Q.shape
    q = (Q @ Wq.T).reshape(b, n, H, D).transpose(0, 2, 1, 3)
    k = (K @ Wk.T).reshape(b, n, H, D).transpose(0, 2, 1, 3)
    v = (V @ Wv.T).reshape(b, n, H, D).transpose(0, 2, 1, 3)
    s = np.einsum("bhnd,bhmd->bhnm", q, k) / tau
    s = np.where(mask == 0, -np.inf, s)
    s = s - s.max(axis=-1, keepdims=True)
    e = np.exp(s)
    a = e / e.sum(axis=-1, keepdims=True)
    o = np.einsum("bhnm,bhmd->bhnd", a, v)
    o = o.transpose(0, 2, 1, 3).reshape(b, n, H * D)
    return (o @ Wo.T + bo).astype(np.float32)


def kernel(**inputs) -> np.ndarray:
    out, _ = _run(inputs, trace=False)
    return out


# revision 79
# speedup vs baseline: 1.1214x; 1.1214x over previous
"""MultiHeadSelfAttention Trainium2 Bass kernel (v3).

Shapes (hardcoded): B=8, N=2048, E=512, H=8 heads, D=64 head dim.
Sharding: data-parallel over batch -> one batch item per NeuronCore (8 cores),
no collectives needed.

Per-core pipeline (bf16 compute, fp32 accumulate):
  prologue (DMA-chasing order Wk,K,Wq,Q0,Wv,V): load f32, cast bf16
        (DVE/ACT), PE-transpose into ^T layouts. k-proj emits kTT[h] =
        [128, N/2]: partitions 0-63 hold head h's d-dims for EVEN key
        tiles, partitions 64-127 for ODD key tiles (col-tiled matmul
        pair with interleaved rhs). q-proj emits qTT[h] with q^T
        duplicated in both partition halves (col-tiled pair) so the
        scores matmuls are ROW-TILED: the two 64-row score matmuls of an
        adjacent key-tile pair run concurrently in the upper/lower
        halves of the PE array. v_aug = V @ Wv^T + per-head ones column
        (softmax denominators ride the PV matmul). Only K/Q-chunk-0/
        V-groups-0,1 gate attention start; the rest streams in as filler.
  attention (qc outer, head inner): per key-tile pair, s2 [128, 2x512] =
        row-tiled score pair; exp on ScalarE for 7/8 pairs, on VectorE
        via a Schraudolph exponent trick (affine -> int16 -> bits
        reinterpreted as bf16) for 1/8. PV is emitted with lag 2 so the
        exp latency is off the critical path: O^T [65, 512] += v_aug^T .
        P^T over 16 key tiles; row 64 = softmax denominators. Remaining
        projections / transposes / previous-chunk output projection are
        interleaved as PE filler (one unit per key-tile pair) to keep
        the HAM clock gate at full rate.
  per-head-pair: denominator row -> DRAM -> broadcast -> fast reciprocal
        (DVE), normalize O^T on gpsimd. After each chunk's last head the
        output projection Y = oTn^T . Wo^T is queued (computed directly
        in [n, e] orientation -- no output transposes), bias-add fused
        into the PSUM->SBUF copy, DMA out; all overlapped with the next
        chunk's attention.

The attention mask is all ones per the problem spec; validated host-side.
"""

import sys

for _p in ("/opt/trn_rl_repo",):
    if _p not in sys.path:
        sys.path.insert(0, _p)

import numpy as np
from collections import deque
from contextlib import ExitStack

import concourse.bass as bass
import concourse.bacc as bacc
import concourse.mybir as mybir
import concourse.tile as tile
from concourse.masks import make_identity

B, N, E = 8, 2048, 512
H, D = 8, 64
P = 128          # partitions
ET = E // P      # 4 e-tiles
NT = N // P      # 16 n-tiles
QC = 512         # q chunk in attention
NQC = N // QC    # 4
KTP = 8          # key-tile pairs; pair k covers key tiles 2k and 2k+1
HV = 65          # head dim + ones column
PVLAG = 3        # PV trails scores by this many key-tile pairs
FP32 = mybir.dt.float32
BF16 = mybir.dt.bfloat16
I16 = mybir.dt.int16
NCORES = 8

# which key-tile pairs' exp goes to the DVE Schraudolph path (rest on ACT)
SCH_KTPS = (2, 6)
# Schraudolph: exp(x) ~ bf16(bits = int16(x*log2(e)*128 + 127*128 + SIGMA))
SCH_SIGMA = -5.0

AF = mybir.ActivationFunctionType
ALU = mybir.AluOpType


def _build(inv_tau: float) -> bass.Bass:
    nc = bacc.Bacc(trn_type="TRN2")

    dQ = nc.dram_tensor("Q", [N, E], FP32, kind="ExternalInput")
    dK = nc.dram_tensor("K", [N, E], FP32, kind="ExternalInput")
    dV = nc.dram_tensor("V", [N, E], FP32, kind="ExternalInput")
    dWq = nc.dram_tensor("Wq", [E, E], FP32, kind="ExternalInput")
    dWk = nc.dram_tensor("Wk", [E, E], FP32, kind="ExternalInput")
    dWv = nc.dram_tensor("Wv", [E, E], FP32, kind="ExternalInput")
    dWo = nc.dram_tensor("Wo", [E, E], FP32, kind="ExternalInput")
    dbo = nc.dram_tensor("bo", [E], FP32, kind="ExternalInput")
    dout = nc.dram_tensor("out", [N, E], FP32, kind="ExternalOutput")
    drs = nc.dram_tensor("r_scratch", [NQC * H * QC], FP32)

    with tile.TileContext(nc) as tc, ExitStack() as ctx:
        _body(ctx, tc, inv_tau, dQ, dK, dV, dWq, dWk, dWv, dWo, dbo, dout, drs)
    nc.finalize()
    return nc


def _body(ctx, tc, inv_tau, dQ, dK, dV, dWq, dWk, dWv, dWo, dbo, dout, drs):
    nc = tc.nc
    dma = nc.sync.dma_start

    const = ctx.enter_context(tc.tile_pool(name="const", bufs=1))
    big = ctx.enter_context(tc.tile_pool(name="big", bufs=1))
    psum = ctx.enter_context(tc.tile_pool(name="psum", bufs=1, space="PSUM"))
    stage = ctx.enter_context(tc.tile_pool(name="stage", bufs=1))
    p2pool = ctx.enter_context(tc.tile_pool(name="p2pool", bufs=4))

    ident = const.tile([P, P], BF16, name="ident", tag="ident")
    make_identity(nc, ident)

    in_attention = [False]

    # ---------------- helpers: loads, casts, transposes, drains ----------
    cast_rr = [0]

    def load_cast(dX, r, fast=False, q2=False):
        """DMA [128,E] f32 slice r, cast to bf16.

        Prologue (or fast=True): DVE/ACT alternating (fast engines, idle
        then). Attention phase: gpsimd (slow but otherwise idle).
        q2: use the second HWDGE queue (scalar) + gpsimd/DVE casts, so V
        streams in parallel with K/Q instead of behind them."""
        x_f32 = stage.tile([P, E], FP32, name="x_f32", tag="x_f32", bufs=4)
        if q2:
            nc.scalar.dma_start(out=x_f32, in_=dX[r * P:(r + 1) * P, :])
            x_bf = stage.tile([P, E], BF16, name="x_bf", tag="x_bf", bufs=8)
            cast_rr[0] ^= 1
            if cast_rr[0]:
                nc.gpsimd.tensor_copy(x_bf, x_f32)
            else:
                nc.vector.tensor_copy(x_bf, x_f32)
            return x_bf
        dma(out=x_f32, in_=dX[r * P:(r + 1) * P, :])
        x_bf = stage.tile([P, E], BF16, name="x_bf", tag="x_bf", bufs=8)
        if in_attention[0] and not fast:
            nc.gpsimd.tensor_copy(x_bf, x_f32)
        else:
            cast_rr[0] ^= 1
            if cast_rr[0]:
                nc.vector.tensor_copy(x_bf, x_f32)
            else:
                nc.scalar.copy(x_bf, x_f32)
        return x_bf

    tp_rr = [0]

    def tp_tile():
        """PSUM staging tile for transposes.

        Prologue rotates over the (idle) s2/o2 tags; during attention the
        pp tag is used instead (s2/o2 are hot in the score/PV pipeline).
        """
        if in_attention[0]:
            return psum.tile([P, E], BF16, name="tp", tag="pp", bufs=2)
        tp_rr[0] ^= 1
        if tp_rr[0]:
            return psum.tile([P, E], BF16, name="tp", tag="s2", bufs=2)
        return psum.tile([P, E], BF16, name="tp", tag="o2", bufs=2)

    copy_rr = [0]

    def drain_copy(out_ap, in_ap):
        """PSUM->SBUF copy, DVE/ACT alternating (ACT has headroom in the
        filler-heavy chunks where these copies bunch up)."""
        copy_rr[0] ^= 1
        if copy_rr[0]:
            nc.scalar.copy(out_ap, in_ap)
        else:
            nc.vector.tensor_copy(out_ap, in_ap)

    # ---------------- SBUF layout ----------------
    # W^T / X^T live in merged tiles (so one XBAR DMA-transpose can write
    # all four e-blocks of an input tile); consumers use slice views.
    # wt[w][c] = [128 in-dims (block c), 512 out-dims]
    # xT[x][et] = [128 e-dims (block et), N]
    wtbig = {w: const.tile([P, ET * E], BF16, name=f"wT_{w}", tag=f"wT_{w}")
             for w in ("q", "k", "v", "o")}
    wt = {w: [wtbig[w][:, c * E:(c + 1) * E] for c in range(ET)]
          for w in ("q", "k", "v", "o")}
    xTbig = {x: big.tile([P, ET * N], BF16, name=f"xT_{x}", tag=f"xT_{x}")
             for x in ("K", "V", "Q")}
    xT = {x: [xTbig[x][:, et * N:(et + 1) * N] for et in range(ET)]
          for x in ("K", "V", "Q")}

    def dma_transpose_tile(dest_big, r, x_bf):
        """One XBAR transpose: x_bf [128, 512] -> 128-col block r of each
        of dest's 4 e-blocks. Used in the attention phase only (the sync
        DMA queue is quiet there; PE is not)."""
        dv = dest_big.rearrange("p (b n) -> p b n", b=ET)
        nc.sync.dma_start_transpose(out=dv[:, :, r * P:(r + 1) * P],
                                    in_=x_bf)
    # kTT[h]: [128, N/2]; col block k = key-tile pair (2k, 2k+1):
    # partitions 0-63 = head h d-dims of tile 2k, 64-127 = tile 2k+1
    kTT = [big.tile([P, N // 2], BF16, name=f"kTT_{h}", tag=f"kTT_{h}")
           for h in range(H)]
    # qTT[h]: [128, N]; q^T duplicated into both partition halves
    qTT = [big.tile([P, N], BF16, name=f"qTT_{h}", tag=f"qTT_{h}")
           for h in range(H)]
    v_aug = [big.tile([P, H * HV], BF16, name=f"vaug_{nt}",
                      tag=f"vaug_{nt}") for nt in range(NT)]

    def ham_heartbeat():
        """One real (non-transpose) matmul: PE-transposes don't count as
        busy for the HAM clock gate, so transpose-heavy stretches would
        re-throttle the PE to 1.2 GHz without these. Needs to fire at
        least every ~2us (the re-throttle window is ~3.4us and a cold
        transpose half-group takes ~2us)."""
        wh = psum.tile([P, P], FP32, name="warm", tag="pp", bufs=2)
        nc.tensor.matmul(wh, lhsT=ident, rhs=ident, start=True, stop=True,
                         skip_group_check=True)

    def emit_w_block(wname, dW, fast=False):
        """Load + cast + transpose one weight matrix into wt[wname]."""
        w_bfs = [load_cast(dW, r, fast) for r in range(ET)]
        for c in range(ET):
            tp = tp_tile()
            for r in range(ET):
                nc.tensor.transpose(
                    tp[:, r * P:(r + 1) * P], w_bfs[r][:, c * P:(c + 1) * P],
                    ident)
            drain_copy(wt[wname][c], tp)
            if not in_attention[0] and c % 2 == 1:
                ham_heartbeat()

    def emit_x_casts(dX, g, fast=False):
        """DMA + cast one group of 4 n-tiles; returns staged bf16 tiles."""
        return [load_cast(dX, g * ET + i, fast) for i in range(ET)]

    def emit_x_transposes(xname, g, x_bfs):
        for et in range(ET):
            tp = tp_tile()
            for i in range(ET):
                nc.tensor.transpose(
                    tp[:, i * P:(i + 1) * P],
                    x_bfs[i][:, et * P:(et + 1) * P], ident)
            drain_copy(xT[xname][et][:, g * E:(g + 1) * E], tp)
            if not in_attention[0] and et % 2 == 1:
                ham_heartbeat()

    def emit_x_group(xname, dX, g, fast=False):
        emit_x_transposes(xname, g, emit_x_casts(dX, g, fast))

    # ---------------- projections ----------------
    def emit_kproj(h, c):
        """col-tiled pair: even key tiles (upper half) + odd (lower half).

        kTT chunk c covers key-tile pairs 4c..4c+3 = tiles 8c..8c+7."""
        pp = psum.tile([P, QC], FP32, name="pp", tag="pp", bufs=2)
        lhs = [wt["k"][et][:, h * D:(h + 1) * D] for et in range(ET)]
        for half in range(2):
            for et in range(ET):
                xk = xT["K"][et].rearrange("p (g u c2) -> p g u c2",
                                           u=2, c2=P)
                nc.tensor.matmul(
                    pp[half * D:(half + 1) * D, :], lhsT=lhs[et],
                    rhs=xk[:, 4 * c:4 * (c + 1), half, :],
                    start=(et == 0), stop=(et == ET - 1))
        drain_copy(kTT[h][:, c * QC:(c + 1) * QC], pp)

    def emit_qproj(h, c):
        """col-tiled pair: same q^T chunk written to both partition halves."""
        pp = psum.tile([P, QC], FP32, name="pp", tag="pp", bufs=2)
        lhs = [wt["q"][et][:, h * D:(h + 1) * D] for et in range(ET)]
        rhs = [xT["Q"][et][:, c * QC:(c + 1) * QC] for et in range(ET)]
        for half in range(2):
            for et in range(ET):
                nc.tensor.matmul(pp[half * D:(half + 1) * D, :],
                                 lhsT=lhs[et], rhs=rhs[et],
                                 start=(et == 0), stop=(et == ET - 1))
        drain_copy(qTT[h][:, c * QC:(c + 1) * QC], pp)

    def emit_vproj(nt):
        pp = psum.tile([P, QC], FP32, name="pp", tag="pp", bufs=2)
        for et in range(ET):
            nc.tensor.matmul(
                pp,
                lhsT=xT["V"][et][:, nt * P:(nt + 1) * P],
                rhs=wt["v"][et],
                start=(et == 0), stop=(et == ET - 1))
        va = v_aug[nt].rearrange("p (h c) -> p h c", c=HV)
        nc.vector.tensor_copy(va[:, :, 0:D], pp.rearrange("p (h d) -> p h d",
                                                          d=D))
        nc.gpsimd.memset(va[:, :, D:HV], 1.0)

    def emit_v_unit(g, x_bfs):
        emit_x_transposes("V", g, x_bfs)
        for nt in range(g * ET, (g + 1) * ET):
            emit_vproj(nt)

    # ---------------- prologue emission (DMA-chasing order) ----------------
    # HAM warmup: dummy matmuls while the first DMAs land (PE would idle
    # anyway; sustained activity lifts the clock gate to 2.4 GHz before
    # the real transposes start). One shared PSUM tile, back-to-back
    # matmuls -- buffer rotation would add gaps and dilute the busy window.
    warm = psum.tile([P, P], FP32, name="warm", tag="pp", bufs=2)
    for _ in range(48):
        nc.tensor.matmul(warm, lhsT=ident, rhs=ident, start=True, stop=True,
                         skip_group_check=True)
    # K group 0 first in the DMA queue (its transposes only need ident).
    # kproj bursts are interleaved between transpose groups: dense matmuls
    # count as PE-busy for the HAM clock gate (transposes don't), keeping
    # the prologue at 2.4 GHz.
    k0_staged = emit_x_casts(dK, 0)
    # bias broadcast to all partitions (consumed only by oproj; queued
    # after the first K tiles so it never delays them)
    bo_sb = const.tile([P, E], FP32, name="bo_sb", tag="bo_sb")
    dma(out=bo_sb, in_=bass.AP(tensor=dbo, offset=0, ap=[[0, P], [1, E]]))
    emit_w_block("k", dWk)
    emit_x_transposes("K", 0, k0_staged)
    for g in range(1, NT // ET):
        emit_x_group("K", dK, g)
    emit_kproj(0, 0)
    emit_kproj(0, 1)
    emit_w_block("q", dWq)
    emit_x_group("Q", dQ, 0)
    emit_qproj(0, 0)
    emit_w_block("v", dWv)
    for g in range(2):
        emit_v_unit(g, emit_x_casts(dV, g))
    # V groups 2,3: DMA+cast now (fast engines), transposes+proj as filler
    v_staged = {g: emit_x_casts(dV, g, fast=True) for g in (2, 3)}

    # ---------------- attention ----------------
    sch_scale = float(inv_tau * np.log2(np.e) * 128.0)
    sch_bias = float(127 * 128 + SCH_SIGMA)

    filler = deque()

    def pop_filler():
        if filler:
            fn, args = filler.popleft()
            fn(*args)

    # seed: per-head k/q projections must be emitted before that head's
    # scores. qc0 pops 4 units per head (ktp 1,3,5,7); the order below is
    # tuned so every unit lands before its first consumer.
    filler.append((emit_v_unit, (2, v_staged[2])))
    filler.append((emit_kproj, (1, 0)))
    filler.append((emit_qproj, (1, 0)))
    filler.append((emit_v_unit, (3, v_staged[3])))
    filler.append((emit_kproj, (1, 1)))
    for h in range(2, H):
        filler.append((emit_kproj, (h, 0)))
        filler.append((emit_qproj, (h, 0)))
        filler.append((emit_kproj, (h, 1)))
        if h in (3, 4, 5):
            filler.append((emit_x_group, ("Q", dQ, h - 2)))
        if h == 6:
            filler.append((emit_w_block, ("o", dWo)))

    def _drs_off(qc, h):
        return qc * H * QC + (h % 4) * 2 * QC + (h // 4) * QC

    def emit_pv(h, o2, p2, ktp):
        for j in range(2):
            kt = 2 * ktp + j
            nc.tensor.matmul(
                o2[:, :],
                lhsT=v_aug[kt][:, h * HV:(h + 1) * HV],
                rhs=p2[:, j * QC:(j + 1) * QC],
                start=(ktp == 0 and j == 0), stop=(ktp == KTP - 1 and j == 1),
                skip_group_check=True)

    rbr_live = {}
    pend = deque()  # (h, o2, p2, ktp, drain_ctx) PVs awaiting emission

    def emit_drains(h, qc, oT_t, oTn_t, l4_t, o2):
        """After head h's last PV: O^T/denominator drains + normalization.

        Drains go to DVE except for the last chunk's final head pair,
        whose tail chain is latency-critical while ACT is winding down
        (earlier qc3 heads stay on DVE -- ACT is the co-pacer there)."""
        drain = (nc.scalar.copy if qc == NQC - 1 and h >= H - 2
                 else nc.vector.tensor_copy)
        hp, h2 = h // 2, (h % 2) * D
        drain(oT_t[hp][h2:h2 + D, :], o2[0:D, :])
        g = 32 * (h % 4)
        l4_slice = l4_t[g:g + 1, (h // 4) * QC:(h // 4 + 1) * QC]
        drain(l4_slice, o2[D:HV, :])
        dma(out=drs[_drs_off(qc, h):_drs_off(qc, h) + QC], in_=l4_slice)
        # broadcast this head's denominators to its 64 partitions
        if h % 2 == 0:
            rbr_live[hp] = stage.tile([P, QC], FP32, name="rbr", tag="rbr",
                                      bufs=2)
        rbr = rbr_live[hp]
        bsrc = bass.AP(tensor=drs, offset=_drs_off(qc, h),
                       ap=[[0, D], [1, QC]])
        dma(out=rbr[h2:h2 + D, :], in_=bsrc)
        if h % 2 == 1:
            # head pair complete: reciprocal + normalize (DVE, so the
            # gpsimd drain queue never blocks on the reciprocal)
            rb = stage.tile([P, QC], FP32, name="rb", tag="rb", bufs=2)
            nc.vector.reciprocal_approx_fast(rb, rbr)
            nc.vector.tensor_tensor(oTn_t[hp], oT_t[hp], rb, ALU.mult)

    def flush_pv():
        h, o2, p2, ktp, drain_ctx = pend.popleft()
        emit_pv(h, o2, p2, ktp)
        if ktp == KTP - 1:
            emit_drains(h, *drain_ctx, o2)

    def attn_head(h, qc, oT_t, oTn_t, l4_t):
        o2 = psum.tile([HV, QC], FP32, name="o2", tag="o2", bufs=2)
        drain_ctx = (qc, oT_t, oTn_t, l4_t)
        for ktp in range(KTP):
            s2 = psum.tile([P, 2 * QC], FP32, name="s2", tag="s2", bufs=2)
            # row-tiled score pair: upper 64 rows = key tile 2*ktp,
            # lower 64 rows = key tile 2*ktp+1
            nc.tensor.matmul(
                s2[:, 0:QC],
                lhsT=kTT[h][0:D, ktp * P:(ktp + 1) * P],
                rhs=qTT[h][0:D, qc * QC:(qc + 1) * QC],
                start=True, stop=True)
            nc.tensor.matmul(
                s2[:, QC:2 * QC],
                lhsT=kTT[h][D:P, ktp * P:(ktp + 1) * P],
                rhs=qTT[h][D:P, qc * QC:(qc + 1) * QC],
                start=True, stop=True)
            p2 = p2pool.tile([P, 2 * QC], BF16, name="p2", tag="p2")
            if ktp in SCH_KTPS:
                nc.vector.tensor_scalar(
                    p2.bitcast(I16), s2, sch_scale, sch_bias,
                    ALU.mult, ALU.add)
            else:
                nc.scalar.activation(p2, s2, AF.Exp, scale=inv_tau)
            pend.append((h, o2, p2, ktp, drain_ctx))
            if len(pend) > PVLAG:
                flush_pv()
            # qc0 pops every other pair (its filler backlog is the whole
            # projection prologue; pacing avoids a PE bulge), later
            # chunks pop every pair. The last chunk's first pops must wait
            # for the previous chunk's PV pipeline (and with it head 7's
            # normalization) to flush out -- its oproj units lead the queue.
            if qc > 0 and h == 0 and ktp < 4:
                pass  # previous chunk's PV pipeline still flushing; its
                # oproj units lead the queue and need head 7's norm
            elif qc > 0 or ktp % 2 == 1:
                pop_filler()

    def emit_oproj(qc, j, oTn_t, phase=0, pp_store=None):
        """y rows nt=qc*4+j: out [128 n, 512 e] = oTn^T blocks . Wo^T.

        phase 0: whole thing. phase 1: e-blocks 0-2 only (available before
        the last head pair is normalized -- fills the tail's norm-chain
        latency). phase 2: e-block 3 + drain + store."""
        if phase != 2:
            tag = "pp" if phase == 0 or j < 2 else ("o2" if j == 2 else "s2")
            pp = psum.tile([P, QC], FP32, name="pp", tag=tag, bufs=2)
            lim = ET if phase == 0 else ET - 1
            for et in range(lim):
                nc.tensor.matmul(
                    pp,
                    lhsT=oTn_t[et][:, j * P:(j + 1) * P],
                    rhs=wt["o"][et],
                    start=(et == 0), stop=(phase == 0 and et == ET - 1),
                    skip_group_check=True)
            if phase == 1:
                pp_store[j] = pp
                return
        else:
            pp = pp_store[j]
            nc.tensor.matmul(
                pp,
                lhsT=oTn_t[ET - 1][:, j * P:(j + 1) * P],
                rhs=wt["o"][ET - 1],
                start=False, stop=True, skip_group_check=True)
        y_sb = stage.tile([P, E], FP32, name="y_sb", tag="y_sb", bufs=2)
        nc.vector.tensor_tensor(y_sb, pp, bo_sb, ALU.add)
        nt = qc * NQC + j
        dma(out=dout[nt * P:(nt + 1) * P, :], in_=y_sb)

    in_attention[0] = True
    for qc in range(NQC):
        l4_t = stage.tile([P, 2 * QC], FP32, name="l4", tag="l4", bufs=2)
        oT_t = [big.tile([P, QC], BF16, name=f"oT_{hp}", tag=f"oT_{hp}",
                         bufs=2) for hp in range(ET)]
        oTn_t = [big.tile([P, QC], BF16, name=f"oTn_{hp}", tag=f"oTn_{hp}",
                          bufs=2) for hp in range(ET)]
        if qc + 1 < NQC:
            for h in range(H):
                filler.append((emit_qproj, (h, qc + 1)))
        for h in range(H):
            attn_head(h, qc, oT_t, oTn_t, l4_t)
        # The PV pipeline spills across the chunk boundary (flushing here
        # would idle the PE on the last exp). This chunk's oproj units are
        # queued behind the next chunk's qproj units, so head 7's drains
        # (emitted when its last PV pops out early next chunk) land first.
        if qc + 1 < NQC:
            for j in range(NQC):
                filler.append((emit_oproj, (qc, j, oTn_t)))
        else:
            while pend:
                flush_pv()
            while filler:
                pop_filler()
            for j in range(NQC):
                emit_oproj(qc, j, oTn_t)


_CACHE = {}


def _get_nc(inv_tau: float) -> bass.Bass:
    key = round(float(inv_tau), 9)
    if key not in _CACHE:
        _CACHE[key] = _build(float(inv_tau))
    return _CACHE[key]


def _run(inputs: dict, trace: bool = False):
    """Returns (output [B,N,E] fp32, BassKernelResults)."""
    from concourse.bass_utils import run_bass_kernel_spmd

    Q = np.ascontiguousarray(np.asarray(inputs["Q"], dtype=np.float32))
    K = np.ascontiguousarray(np.asarray(inputs["K"], dtype=np.float32))
    V = np.ascontiguousarray(np.asarray(inputs["V"], dtype=np.float32))
    Wq = np.ascontiguousarray(np.asarray(inputs["Wq"], dtype=np.float32))
    Wk = np.ascontiguousarray(np.asarray(inputs["Wk"], dtype=np.float32))
    Wv = np.ascontiguousarray(np.asarray(inputs["Wv"], dtype=np.float32))
    Wo = np.ascontiguousarray(np.asarray(inputs["Wo"], dtype=np.float32))
    bo = np.ascontiguousarray(np.asarray(inputs["bo"], dtype=np.float32))
    tau = float(np.asarray(inputs["tau"]))

    mask = inputs.get("attn_mask")
    if mask is not None and not np.all(np.asarray(mask) != 0):
        # Fallback (never hit for the spec'd all-ones mask): host math.
        return _host_reference(Q, K, V, np.asarray(mask), Wq, Wk, Wv, Wo,
                               bo, tau), None

    nc = _get_nc(1.0 / tau)
    in_maps = []
    for b in range(NCORES):
        in_maps.append({
            "Q": Q[b], "K": K[b], "V": V[b],
            "Wq": Wq, "Wk": Wk, "Wv": Wv, "Wo": Wo, "bo": bo,
        })
    res = run_bass_kernel_spmd(nc, in_maps, list(range(NCORES)), trace=trace)
    out = np.stack([np.asarray(res.results[b]["out"]) for b in range(NCORES)])
    return out.astype(np.float32), res


def _host_reference(Q, K, V, mask, Wq, Wk, Wv, Wo, bo, tau):
    b, n, _ = Q.shape
    q = (Q @ Wq.T).reshape(b, n, H, D).transpose(0, 2, 1, 3)
    k = (K @ Wk.T).reshape(b, n, H, D).transpose(0, 2, 1, 3)
    v = (V @ Wv.T).reshape(b, n, H, D).transpose(0, 2, 1, 3)
    s = np.einsum("bhnd,bhmd->bhnm", q, k) / tau
    s = np.where(mask == 0, -np.inf, s)
    s = s - s.max(axis=-1, keepdims=True)
    e = np.exp(s)
    a = e / e.sum(axis=-1, keepdims=True)
    o = np.einsum("bhnm,bhmd->bhnd", a, v)
    o = o.transpose(0, 2, 1, 3).reshape(b, n, H * D)
    return (o @ Wo.T + bo).astype(np.float32)


def kernel(**inputs) -> np.ndarray:
    out, _ = _run(inputs, trace=False)
    return out
